# revision 16
# baseline (speedup 1.0000x reference)
"""Causal self-attention on 8 Trainium2 NeuronCores.

Sharding (batch + head parallel): core c handles batch b = c // 4 and the
4 heads [hg*4, hg*4+4) where hg = c % 4.  Each core computes q/k/v from
column-sliced c_attn weights, full causal attention for its heads, and a
partial c_proj output from the matching row slice of w_proj; the host sums
the 4 partials per batch.

Matmuls run in bf16 (fp32 PSUM accumulate), except the q/k projections
which run in fp8-e4m3 with DoubleRow perf mode (two k-tiles per matmul,
2 elem/cell/cycle); wq/wk are pre-scaled by 32 to clear the e4m3
subnormal range and the 1/(32*32) is folded into the softmax scale.
The two heads of a pair
are computed CONCURRENTLY in the PE array for the S = K^T Q matmuls via
row tiling (K=64 contraction each, tile_position rows 0-63 / 64-127); S
batches of two key blocks keep the PE in 64-row tiling mode back-to-back
so the mode-switch drain is paid once per batch.  The causal mask is a
0/1 triu multiply applied to the exp() output on the (otherwise idle)
GPSIMD engine, so the PE only runs the productive matmul streams; the
softmax denominator is broadcast across partitions with a bf16 ones
matmul (vs f32r in earlier revisions - ~3x faster on the PE).

Emission WEAVES attention chunks with independent PE filler work (the
next stage's qkv matmul groups and c_proj halves) so the ACT exp stream
- the second-busiest engine - runs from ~10us in, and the PE always has
independent work queued behind ACT-dependent chunks (no head-of-line
stalls, HAM clock-gate stays at 2.4 GHz).  The first x stage and wqk are
DMAd in interleaved per-kt chunks so the first qkv matmul only waits for
one 256KB pair instead of the full 2MB.
"""

import sys

if "/opt/trn_rl_repo" not in sys.path:
    sys.path.insert(0, "/opt/trn_rl_repo")

import numpy as np

import concourse.mybir as mybir
from concourse import bacc
from concourse.bass_utils import run_bass_kernel_spmd
from concourse.tile import TileContext

B, T, C = 2, 2048, 1024
H, D = 16, 64
HL = 4  # heads per core
N_CORES = 8
KT = C // 128  # contraction tiles over the embedding dim
WS = 32.0  # fp8 pre-scale on wq/wk (keeps them out of e4m3 subnormals)
SCALE = 1.0 / (8.0 * WS * WS)  # 1/sqrt(D), with the q&k pre-scales folded in

_CACHE = {}


def _build():
    f32 = mybir.dt.float32
    bf16 = mybir.dt.bfloat16
    fp8 = mybir.dt.float8e4
    DR = mybir.MatmulPerfMode.DoubleRow
    nc = bacc.Bacc("TRN2", target_bir_lowering=False, debug=False, num_devices=N_CORES)

    x_in = nc.dram_tensor("x_in", [128, 4, KT, T // 4], bf16, kind="ExternalInput")
    x8_in = nc.dram_tensor(
        "x8_in", [128, 4, KT // 2, 2, T // 4], fp8, kind="ExternalInput"
    )
    wqk8 = nc.dram_tensor(
        "wqk8", [128, KT // 2, 2, 2 * HL * D], fp8, kind="ExternalInput"
    )
    wv = nc.dram_tensor("wv", [128, KT, HL * D], bf16, kind="ExternalInput")
    wp = nc.dram_tensor("wp", [128, HL // 2, C], bf16, kind="ExternalInput")
    # consts: two copies of triu01 (1 where k <= q) for the two s slices
    consts = nc.dram_tensor("consts", [128, 256], bf16, kind="ExternalInput")
    out = nc.dram_tensor("out", [T, C], bf16, kind="ExternalOutput")

    EXP = mybir.ActivationFunctionType.Exp

    with TileContext(nc) as tc:
        with tc.tile_pool(name="persist", bufs=1) as persist:
            # q/k feature-major [d, t]: slot 0/1 = q head-pairs 0/1, 2/3 = k;
            # partitions 0-63 = even head dims, 64-127 = odd head dims.
            qk_t = [
                [persist.tile([128, 512], bf16, name=f"qk{s}_{tb}") for tb in range(4)]
                for s in range(4)
            ]
            # v token-major per 128-token tile; col D holds ones (denominator)
            v_t = [
                persist.tile([128, HL, D + 1], bf16, name=f"v{tt}") for tt in range(16)
            ]
            # head-pair stacked normalized y per 512-token block
            y2_t = [
                persist.tile([128, HL // 2, 512], bf16, name=f"y2{b_}")
                for b_ in range(4)
            ]
            wp_sb = persist.tile([128, HL // 2, C], bf16)
            consts_sb = persist.tile([128, 256], bf16)
            triu2 = consts_sb[:, :].rearrange("p (s q) -> p s q", s=2)
            ones_sb = persist.tile([128, 64], bf16)
            # denominator staging rows: all partitions zero except row D, so
            # the broadcast matmul can contract K=128 (full-array mode, no
            # tiling-mode switch/drain on the PE)
            rsb_slots = [
                persist.tile([128, 2, 512], bf16, name=f"rsb{i}") for i in range(2)
            ]
            for r_ in rsb_slots:
                nc.gpsimd.memset(r_.bitcast(mybir.dt.uint16), 0)
            nc.gpsimd.memset(ones_sb, 1.0)

            for tt in range(16):
                nc.gpsimd.memset(v_t[tt][:, :, D : D + 1], 1.0)

            with (
                tc.tile_pool(name="qkvp", bufs=1) as qkvp,
                tc.tile_pool(name="attp", bufs=10) as attp,
                tc.tile_pool(name="attsmall", bufs=3) as attsmall,
                tc.tile_pool(name="projp", bufs=3) as projp,
                tc.tile_pool(name="ps_st", bufs=2, space="PSUM") as ps_st,
                tc.tile_pool(name="ps_y", bufs=1, space="PSUM") as ps_y,
                tc.tile_pool(name="ps_share", bufs=2, space="PSUM") as ps_share,
            ):
                # quarter-length x buffers, double-buffered across stages
                x_q = [
                    qkvp.tile([128, KT, T // 4], bf16, name=f"x_q{i}") for i in range(2)
                ]
                x8_q = [
                    qkvp.tile([128, KT // 2, 2, T // 4], fp8, name=f"x8_q{i}")
                    for i in range(2)
                ]
                wqk8_sb = qkvp.tile([128, KT // 2, 2, 2 * HL * D], fp8)
                wv_sb = qkvp.tile([128, KT, HL * D], bf16)

                def stage_dma(tb):
                    # stage-major x layout: 8KB contiguous per partition ->
                    # few large DMA descriptors instead of ~1K small ones
                    nc.sync.dma_start(x8_q[tb % 2], x8_in[:, tb])
                    nc.sync.dma_start(x_q[tb % 2], x_in[:, tb])

                def qk_group(tb, jt):
                    def go():
                        # fp8 DoubleRow: two 128-deep k-tiles per matmul at
                        # 2 elem/cell/cycle - ~1.4x the bf16 qk throughput
                        x8 = x8_q[tb % 2]
                        qk_ps = ps_share.tile(
                            [128, 512], f32, tag="share", name="qk_ps"
                        )
                        for t in range(KT // 2):
                            nc.tensor.matmul(
                                qk_ps,
                                wqk8_sb[:, t, :, jt * 128 : (jt + 1) * 128],
                                x8[:, t],
                                start=(t == 0),
                                stop=(t == KT // 2 - 1),
                                perf_mode=DR,
                            )
                        nc.vector.tensor_copy(qk_t[jt][tb], qk_ps)

                    return go

                def v_group(tb, tt2):
                    def go():
                        x_sb = x_q[tb % 2]
                        tt = tb * 4 + tt2
                        v_ps = ps_share.tile(
                            [128, HL * D], f32, tag="share", name="v_ps"
                        )
                        for kt in range(KT):
                            nc.tensor.matmul(
                                v_ps,
                                x_sb[:, kt, tt2 * 128 : (tt2 + 1) * 128],
                                wv_sb[:, kt, :],
                                start=(kt == 0),
                                stop=(kt == KT - 1),
                            )
                        nc.vector.tensor_copy(
                            v_t[tt][:, :, 0:D],
                            v_ps.rearrange("p (h d) -> p h d", h=HL),
                        )

                    return go

                def stage_groups(tb):
                    # zipped so q and k/v land evenly through the row
                    return [qk_group(tb, 0), qk_group(tb, 1), v_group(tb, 0),
                            qk_group(tb, 2), v_group(tb, 1), qk_group(tb, 3),
                            v_group(tb, 2), v_group(tb, 3)]

                def proj_tile(blk, tt):
                    def go():
                        o_sb = projp.tile([128, C], bf16, name="o_sb")
                        off = (tt % 4) * 128
                        for cb in range(2):
                            o_ps = ps_share.tile(
                                [128, 512], f32, tag="share", name="o_ps"
                            )
                            for pr in range(2):
                                nc.tensor.matmul(
                                    o_ps,
                                    y2_t[blk][:, pr, off : off + 128],
                                    wp_sb[:, pr, cb * 512 : (cb + 1) * 512],
                                    start=(pr == 0),
                                    stop=(pr == 1),
                                )
                            # the last block runs after the exp stream ends:
                            # evacuate on ACT and DVE alternately so neither
                            # engine serializes the final matmul stream
                            if blk == 3:
                                if cb == 0:
                                    nc.scalar.copy(o_sb[:, 0:512], o_ps)
                                else:
                                    nc.vector.tensor_copy(o_sb[:, 512:1024], o_ps)
                                nc.sync.dma_start(
                                    out[tt * 128 : (tt + 1) * 128,
                                        cb * 512 : (cb + 1) * 512],
                                    o_sb[:, cb * 512 : (cb + 1) * 512],
                                )
                            else:
                                nc.vector.tensor_copy(
                                    o_sb[:, cb * 512 : (cb + 1) * 512], o_ps
                                )
                        if blk != 3:
                            nc.sync.dma_start(out[tt * 128 : (tt + 1) * 128, :], o_sb)

                    return go

                def attention_chunks(jq):
                    njt = 4 * (jq + 1)
                    chunks = []

                    def build_hp(hp):
                        # separate scope per hp: closures run later (in the
                        # weave), so they must bind this hp's values now
                        h0, h1 = 2 * hp, 2 * hp + 1
                        kslot, qslot = 2 + hp, hp
                        state = {"ests": [None] * njt}

                        def s_pair(j, state):
                            w = max(0, (j - 4 * jq) * 128)
                            st = ps_st.tile([128, 2, 512], f32, name="st")
                            for s, pbase in ((0, 0), (1, 64)):
                                nc.tensor.matmul(
                                    st[:, s, w:],
                                    qk_t[kslot][j // 4][
                                        pbase : pbase + D,
                                        (j % 4) * 128 : (j % 4 + 1) * 128,
                                    ],
                                    qk_t[qslot][jq][pbase : pbase + D, w:],
                                    start=True,
                                    stop=True,
                                    tile_position=(pbase, 0),
                                )
                            return st

                        def exp_block(j, st):
                            w = max(0, (j - 4 * jq) * 128)
                            est = attp.tile(
                                [128, 2, 512], bf16, tag="est", name="est"
                            )
                            nc.scalar.activation(
                                est[:, :, w:], st[:, :, w:], EXP, scale=SCALE
                            )
                            if j >= 4 * jq:
                                # causal mask: zero est where k > q inside the
                                # diagonal 128-block, on the idle GPSIMD
                                nc.gpsimd.tensor_mul(
                                    est[:, :, w : w + 128],
                                    est[:, :, w : w + 128],
                                    triu2,
                                )
                            return est

                        def pv_block(j, state):
                            w = max(0, (j - 4 * jq) * 128)
                            for s, h in ((0, h0), (1, h1)):
                                nc.tensor.matmul(
                                    state["y2ps"][:, s, w:],
                                    v_t[j][:, h, :],
                                    state["ests"][j][:, s, w:],
                                    start=(j == 0),
                                    stop=(j == njt - 1),
                                )

                        def batch(jb, state=state):
                            def go():
                                if jb == 0:
                                    state["y2ps"] = ps_y.tile(
                                        [D + 1, 2, 512], f32, name="y2ps"
                                    )
                                stA = s_pair(jb, state)
                                stB = s_pair(jb + 1, state)
                                state["ests"][jb] = exp_block(jb, stA)
                                state["ests"][jb + 1] = exp_block(jb + 1, stB)
                                if jb >= 4:
                                    pv_block(jb - 4, state)
                                    pv_block(jb - 3, state)

                            return go

                        def tail(state=state, hp=hp):
                            def go():
                                for j_ in range(max(0, njt - 4), njt):
                                    pv_block(j_, state)
                                # epi part 1: denominator row -> bf16 sbuf
                                rsb = rsb_slots[(2 * jq + hp) % 2]
                                nc.vector.tensor_copy(
                                    rsb[D : D + 1, :, :],
                                    state["y2ps"][D : D + 1, :, :],
                                )

                            return go

                        def epi2(state=state, hp=hp):
                            def go():
                                # K=128 bf16 broadcast (full-array mode) ->
                                # DVE reciprocal -> psum-by-sbuf normalize.
                                rsb = rsb_slots[(2 * jq + hp) % 2]
                                y2ps = state["y2ps"]
                                rb_sb = attsmall.tile([D, 2, 512], f32, tag="rbs")
                                for s in range(2):
                                    rb_ps = ps_share.tile(
                                        [D, 512], f32, tag="share", name=f"rb{s}"
                                    )
                                    nc.tensor.matmul(
                                        rb_ps,
                                        ones_sb[:, :],
                                        rsb[:, s, :],
                                        start=True,
                                        stop=True,
                                    )
                                    nc.vector.reciprocal_approx_fast(
                                        rb_sb[:, s, :], rb_ps
                                    )
                                nc.vector.tensor_mul(
                                    y2_t[jq][0:D, hp, :],
                                    y2ps[0:D, 0, :],
                                    rb_sb[:, 0, :],
                                )
                                y_lo = attsmall.tile([D, 512], bf16, tag="ylo")
                                nc.vector.tensor_mul(
                                    y_lo, y2ps[0:D, 1, :], rb_sb[:, 1, :]
                                )
                                nc.gpsimd.dma_start(
                                    y2_t[jq][D:128, hp, :], y_lo
                                )

                            return go

                        for jb in range(0, njt, 2):
                            chunks.append(batch(jb))
                        chunks.append(tail())
                        chunks.append(epi2())

                    for hp in range(2):
                        build_hp(hp)
                    return chunks

                def weave(chunks, fillers):
                    n, m = len(chunks), len(fillers)
                    fi = 0
                    for i, ch in enumerate(chunks):
                        ch()
                        want = (i + 1) * m // n
                        while fi < want:
                            fillers[fi]()
                            fi += 1

                # DMA order matters for the kernel head: consts go first so
                # the PE can run warm-up matmuls (HAM un-throttle) during the
                # x8/wqk8 wait; those are split in interleaved halves so the
                # first qkv group only waits for its own contraction slices.
                HKT = KT // 4
                # first-chunk x8/wqk8 go through the gpsimd SWDGE queue so
                # their ~600ns issue slots run in parallel with the sync
                # queue's instead of behind them
                nc.gpsimd.dma_start(x8_q[0][:, 0:HKT], x8_in[:, 0, 0:HKT])
                nc.gpsimd.dma_start(wqk8_sb[:, 0:HKT], wqk8[:, 0:HKT])
                nc.sync.dma_start(consts_sb, consts[:, :])
                nc.sync.dma_start(x8_q[0][:, HKT:], x8_in[:, 0, HKT:])
                nc.sync.dma_start(wqk8_sb[:, HKT:], wqk8[:, HKT:])
                nc.sync.dma_start(x_q[0], x_in[:, 0])
                nc.sync.dma_start(wv_sb, wv[:, :, :])
                nc.sync.dma_start(wp_sb, wp[:, :, :])

                # PE warm-up: ~1.5us of throwaway matmuls on the consts tile
                # while the first x8/wqk8 chunks are still in flight, so the
                # HAM clock gate opens to 2.4 GHz before the real work lands
                warm_ps = ps_share.tile([128, 256], f32, tag="share")
                for _ in range(6):
                    nc.tensor.matmul(
                        warm_ps, consts_sb[:, 0:128], consts_sb[:, 0:256],
                        start=True, stop=True,
                    )

                # head: interleave stage-0 qkv with row-0 attention so the
                # ACT exp stream (the second-longest engine resource) starts
                # as soon as hp0's q and k are in SBUF instead of after all
                # eight qkv groups
                g0 = stage_groups(0)  # [qk0, qk1, v0, qk2, v1, qk3, v2, v3]
                c0 = attention_chunks(0)
                for step in (g0[0], g0[3], c0[0], g0[1], g0[5], c0[1],
                             g0[2], g0[4], g0[6], g0[7]):
                    step()
                stage_dma(1)
                weave(c0[2:], stage_groups(1))
                # rows 1-2 are woven with the next stage's qkv groups and the
                # previous block's c_proj as PE filler; row 3 (ACT-bound: the
                # longest exp stream) gets stage 3's v groups as extra PE
                # filler, front-loaded so they land before hp0's pv tail
                for tb in (1, 2):
                    g_next = stage_groups(tb + 1)
                    stage_dma(tb + 1)
                    fillers = g_next if tb == 1 else [g_next[i] for i in (0, 1, 3, 5)]
                    fillers += [proj_tile(tb - 1, 4 * (tb - 1) + i)
                                for i in range(4)]
                    weave(attention_chunks(tb), fillers)
                g3v = [stage_groups(3)[i] for i in (2, 4, 6, 7)]
                c3 = attention_chunks(3)
                weave(c3[0:8], g3v)
                weave(c3[8:], [proj_tile(2, 8 + i) for i in range(4)])
                for i in range(4):
                    proj_tile(3, 12 + i)()

    nc.compile()
    return nc


def _get_nc():
    if "nc" not in _CACHE:
        _CACHE["nc"] = _build()
    return _CACHE["nc"]


def make_in_maps(x, w_attn, w_proj):
    import ml_dtypes

    bf16 = ml_dtypes.bfloat16
    fp8 = getattr(ml_dtypes, "float8_e4m3fn", None) or ml_dtypes.float8_e4m3
    x = np.asarray(x, np.float32)
    w_attn = np.asarray(w_attn, np.float32)
    w_proj = np.asarray(w_proj, np.float32)

    # 1 where k <= q (keep), 0 where k > q (causal-masked), twice for s=0/1
    triu01 = np.triu(np.ones((128, 128), np.float32), 0)
    consts = np.concatenate([triu01, triu01], axis=1).astype(bf16)

    in_maps = []
    for c in range(N_CORES):
        b, hg = c // 4, c % 4
        hs = hg * HL * D  # 256 * hg
        xt = np.ascontiguousarray(x[b].T)  # [C, T]
        # [128, tb, KT, 512]: per-partition-contiguous per stage
        x_t = xt.reshape(KT, 128, 4, T // 4).transpose(1, 2, 0, 3)
        x_t = np.ascontiguousarray(x_t)
        x8_t = x_t.reshape(128, 4, KT // 2, 2, T // 4)
        wq = w_attn[hs : hs + HL * D, :]
        wk = w_attn[C + hs : C + hs + HL * D, :]
        wqkt = np.concatenate([wq, wk], 0).T  # [C, 512]
        wqk_t = np.ascontiguousarray(
            wqkt.reshape(KT, 128, 2 * HL * D).transpose(1, 0, 2)
        )
        wqk8_t = (wqk_t * WS).reshape(128, KT // 2, 2, 2 * HL * D)
        wvt = w_attn[2 * C + hs : 2 * C + hs + HL * D, :].T  # [C, 256]
        wv_t = wvt.reshape(KT, 128, HL * D).transpose(1, 0, 2)
        # head-pair stacked rows: [128, HL//2, C]; partition p of pair pr is
        # local feature pr*128 + p (head 2*pr dims then head 2*pr+1 dims)
        wp_t = (
            w_proj[:, hs : hs + HL * D].T.reshape(HL // 2, 128, C).transpose(1, 0, 2)
        )
        in_maps.append(
            {
                "x_in": x_t.astype(bf16),
                "x8_in": x8_t.astype(bf16).astype(fp8),
                "wqk8": wqk8_t.astype(bf16).astype(fp8),
                "wv": np.ascontiguousarray(wv_t).astype(bf16),
                "wp": np.ascontiguousarray(wp_t).astype(bf16),
                "consts": consts,
            }
        )
    return in_maps


def run(in_maps, **kwargs):
    nc = _get_nc()
    return run_bass_kernel_spmd(nc, in_maps, core_ids=list(range(N_CORES)), **kwargs)


def combine(results):
    out = np.zeros((B, T, C), np.float64)
    for c in range(N_CORES):
        out[c // 4] += results[c]["out"].astype(np.float64)
    return out.astype(np.float32)


def kernel(x, w_attn, w_proj):
    res = run(make_in_maps(x, w_attn, w_proj))
    return combine(res.results)


# revision 19
# speedup vs baseline: 1.0067x; 1.0067x over previous
"""Causal self-attention on 8 Trainium2 NeuronCores.

Sharding (batch + head parallel): core c handles batch b = c // 4 and the
4 heads [hg*4, hg*4+4) where hg = c % 4.  Each core computes q/k/v from
column-sliced c_attn weights, full causal attention for its heads, and a
partial c_proj output from the matching row slice of w_proj; the host sums
the 4 partials per batch.

Matmuls run in bf16 (fp32 PSUM accumulate), except the q/k projections
which run in fp8-e4m3 with DoubleRow perf mode (two k-tiles per matmul,
2 elem/cell/cycle); wq/wk are pre-scaled by 32 to clear the e4m3
subnormal range and the 1/(32*32) is folded into the softmax scale.
The two heads of a pair
are computed CONCURRENTLY in the PE array for the S = K^T Q matmuls via
row tiling (K=64 contraction each, tile_position rows 0-63 / 64-127); S
batches of two key blocks keep the PE in 64-row tiling mode back-to-back
so the mode-switch drain is paid once per batch.  The causal mask is a
0/1 triu multiply applied to the exp() output on the (otherwise idle)
GPSIMD engine, so the PE only runs the productive matmul streams; the
softmax denominator is broadcast across partitions with a bf16 ones
matmul (vs f32r in earlier revisions - ~3x faster on the PE).

Emission WEAVES attention chunks with independent PE filler work (the
next stage's qkv matmul groups and c_proj halves) so the ACT exp stream
- the second-busiest engine - runs from ~10us in, and the PE always has
independent work queued behind ACT-dependent chunks (no head-of-line
stalls, HAM clock-gate stays at 2.4 GHz).  The first x stage and wqk are
DMAd in interleaved per-kt chunks so the first qkv matmul only waits for
one 256KB pair instead of the full 2MB.
"""

import sys

if "/opt/trn_rl_repo" not in sys.path:
    sys.path.insert(0, "/opt/trn_rl_repo")

import numpy as np

import concourse.mybir as mybir
from concourse import bacc
from concourse.bass_utils import run_bass_kernel_spmd
from concourse.tile import TileContext

B, T, C = 2, 2048, 1024
H, D = 16, 64
HL = 4  # heads per core
N_CORES = 8
KT = C // 128  # contraction tiles over the embedding dim
WS = 32.0  # fp8 pre-scale on wq/wk (keeps them out of e4m3 subnormals)
SCALE = 1.0 / (8.0 * WS * WS)  # 1/sqrt(D), with the q&k pre-scales folded in

_CACHE = {}


def _build():
    f32 = mybir.dt.float32
    bf16 = mybir.dt.bfloat16
    fp8 = mybir.dt.float8e4
    DR = mybir.MatmulPerfMode.DoubleRow
    nc = bacc.Bacc("TRN2", target_bir_lowering=False, debug=False, num_devices=N_CORES)

    x_in = nc.dram_tensor("x_in", [128, 4, KT, T // 4], bf16, kind="ExternalInput")
    x8_in = nc.dram_tensor(
        "x8_in", [128, 4, KT // 2, 2, T // 4], fp8, kind="ExternalInput"
    )
    wqk8 = nc.dram_tensor(
        "wqk8", [128, KT // 2, 2, 2 * HL * D], fp8, kind="ExternalInput"
    )
    wv = nc.dram_tensor("wv", [128, KT, HL * D], bf16, kind="ExternalInput")
    wp = nc.dram_tensor("wp", [128, HL // 2, C], bf16, kind="ExternalInput")
    # consts: two copies of triu01 (1 where k <= q) for the two s slices
    consts = nc.dram_tensor("consts", [128, 256], bf16, kind="ExternalInput")
    out = nc.dram_tensor("out", [T, C], bf16, kind="ExternalOutput")

    EXP = mybir.ActivationFunctionType.Exp

    with TileContext(nc) as tc:
        with tc.tile_pool(name="persist", bufs=1) as persist:
            # q/k feature-major [d, t]: slot 0/1 = q head-pairs 0/1, 2/3 = k;
            # partitions 0-63 = even head dims, 64-127 = odd head dims.
            qk_t = [
                [persist.tile([128, 512], bf16, name=f"qk{s}_{tb}") for tb in range(4)]
                for s in range(4)
            ]
            # v token-major per 128-token tile; col D holds ones (denominator)
            v_t = [
                persist.tile([128, HL, D + 1], bf16, name=f"v{tt}") for tt in range(16)
            ]
            # head-pair stacked normalized y per 512-token block
            y2_t = [
                persist.tile([128, HL // 2, 512], bf16, name=f"y2{b_}")
                for b_ in range(4)
            ]
            wp_sb = persist.tile([128, HL // 2, C], bf16)
            consts_sb = persist.tile([128, 256], bf16)
            triu2 = consts_sb[:, :].rearrange("p (s q) -> p s q", s=2)
            ones_sb = persist.tile([128, 64], bf16)
            # denominator staging rows: all partitions zero except row D, so
            # the broadcast matmul can contract K=128 (full-array mode, no
            # tiling-mode switch/drain on the PE)
            rsb_slots = [
                persist.tile([128, 2, 512], bf16, name=f"rsb{i}") for i in range(2)
            ]
            for r_ in rsb_slots:
                nc.gpsimd.memset(r_.bitcast(mybir.dt.uint16), 0)
            nc.gpsimd.memset(ones_sb, 1.0)

            for tt in range(16):
                nc.gpsimd.memset(v_t[tt][:, :, D : D + 1], 1.0)

            with (
                tc.tile_pool(name="qkvp", bufs=1) as qkvp,
                tc.tile_pool(name="attp", bufs=10) as attp,
                tc.tile_pool(name="attsmall", bufs=3) as attsmall,
                tc.tile_pool(name="projp", bufs=3) as projp,
                tc.tile_pool(name="ps_st", bufs=2, space="PSUM") as ps_st,
                tc.tile_pool(name="ps_y", bufs=1, space="PSUM") as ps_y,
                tc.tile_pool(name="ps_share", bufs=2, space="PSUM") as ps_share,
            ):
                # quarter-length x buffers, double-buffered across stages
                x_q = [
                    qkvp.tile([128, KT, T // 4], bf16, name=f"x_q{i}") for i in range(2)
                ]
                x8_q = [
                    qkvp.tile([128, KT // 2, 2, T // 4], fp8, name=f"x8_q{i}")
                    for i in range(2)
                ]
                wqk8_sb = qkvp.tile([128, KT // 2, 2, 2 * HL * D], fp8)
                wv_sb = qkvp.tile([128, KT, HL * D], bf16)

                def stage_dma(tb):
                    # stage-major x layout: 8KB contiguous per partition ->
                    # few large DMA descriptors instead of ~1K small ones
                    nc.sync.dma_start(x8_q[tb % 2], x8_in[:, tb])
                    nc.sync.dma_start(x_q[tb % 2], x_in[:, tb])

                def qk_group(tb, jt):
                    def go():
                        # fp8 DoubleRow: two 128-deep k-tiles per matmul at
                        # 2 elem/cell/cycle - ~1.4x the bf16 qk throughput
                        x8 = x8_q[tb % 2]
                        qk_ps = ps_share.tile(
                            [128, 512], f32, tag="share", name="qk_ps"
                        )
                        for t in range(KT // 2):
                            nc.tensor.matmul(
                                qk_ps,
                                wqk8_sb[:, t, :, jt * 128 : (jt + 1) * 128],
                                x8[:, t],
                                start=(t == 0),
                                stop=(t == KT // 2 - 1),
                                perf_mode=DR,
                            )
                        nc.vector.tensor_copy(qk_t[jt][tb], qk_ps)

                    return go

                def v_group(tb, tt2):
                    def go():
                        x_sb = x_q[tb % 2]
                        tt = tb * 4 + tt2
                        v_ps = ps_share.tile(
                            [128, HL * D], f32, tag="share", name="v_ps"
                        )
                        for kt in range(KT):
                            nc.tensor.matmul(
                                v_ps,
                                x_sb[:, kt, tt2 * 128 : (tt2 + 1) * 128],
                                wv_sb[:, kt, :],
                                start=(kt == 0),
                                stop=(kt == KT - 1),
                            )
                        nc.vector.tensor_copy(
                            v_t[tt][:, :, 0:D],
                            v_ps.rearrange("p (h d) -> p h d", h=HL),
                        )

                    return go

                def stage_groups(tb):
                    # zipped so q and k/v land evenly through the row
                    return [qk_group(tb, 0), qk_group(tb, 1), v_group(tb, 0),
                            qk_group(tb, 2), v_group(tb, 1), qk_group(tb, 3),
                            v_group(tb, 2), v_group(tb, 3)]

                def proj_tile(blk, tt):
                    def go():
                        o_sb = projp.tile([128, C], bf16, name="o_sb")
                        off = (tt % 4) * 128
                        for cb in range(2):
                            o_ps = ps_share.tile(
                                [128, 512], f32, tag="share", name="o_ps"
                            )
                            for pr in range(2):
                                nc.tensor.matmul(
                                    o_ps,
                                    y2_t[blk][:, pr, off : off + 128],
                                    wp_sb[:, pr, cb * 512 : (cb + 1) * 512],
                                    start=(pr == 0),
                                    stop=(pr == 1),
                                )
                            # the last block runs after the exp stream ends:
                            # evacuate on ACT and DVE alternately so neither
                            # engine serializes the final matmul stream
                            if blk == 3:
                                if cb == 0:
                                    nc.scalar.copy(o_sb[:, 0:512], o_ps)
                                else:
                                    nc.vector.tensor_copy(o_sb[:, 512:1024], o_ps)
                                nc.sync.dma_start(
                                    out[tt * 128 : (tt + 1) * 128,
                                        cb * 512 : (cb + 1) * 512],
                                    o_sb[:, cb * 512 : (cb + 1) * 512],
                                )
                            else:
                                nc.vector.tensor_copy(
                                    o_sb[:, cb * 512 : (cb + 1) * 512], o_ps
                                )
                        if blk != 3:
                            nc.sync.dma_start(out[tt * 128 : (tt + 1) * 128, :], o_sb)

                    return go

                def attention_chunks(jq):
                    njt = 4 * (jq + 1)
                    chunks = []

                    def build_hp(hp):
                        # separate scope per hp: closures run later (in the
                        # weave), so they must bind this hp's values now
                        h0, h1 = 2 * hp, 2 * hp + 1
                        kslot, qslot = 2 + hp, hp
                        state = {"ests": [None] * njt}

                        def s_pair(j, state):
                            w = max(0, (j - 4 * jq) * 128)
                            st = ps_st.tile([128, 2, 512], f32, name="st")
                            for s, pbase in ((0, 0), (1, 64)):
                                nc.tensor.matmul(
                                    st[:, s, w:],
                                    qk_t[kslot][j // 4][
                                        pbase : pbase + D,
                                        (j % 4) * 128 : (j % 4 + 1) * 128,
                                    ],
                                    qk_t[qslot][jq][pbase : pbase + D, w:],
                                    start=True,
                                    stop=True,
                                    tile_position=(pbase, 0),
                                )
                            return st

                        def exp_block(j, st):
                            w = max(0, (j - 4 * jq) * 128)
                            est = attp.tile(
                                [128, 2, 512], bf16, tag="est", name="est"
                            )
                            nc.scalar.activation(
                                est[:, :, w:], st[:, :, w:], EXP, scale=SCALE
                            )
                            if j >= 4 * jq:
                                # causal mask: zero est where k > q inside the
                                # diagonal 128-block, on the idle GPSIMD
                                nc.gpsimd.tensor_mul(
                                    est[:, :, w : w + 128],
                                    est[:, :, w : w + 128],
                                    triu2,
                                )
                            return est

                        def pv_block(j, state):
                            w = max(0, (j - 4 * jq) * 128)
                            for s, h in ((0, h0), (1, h1)):
                                nc.tensor.matmul(
                                    state["y2ps"][:, s, w:],
                                    v_t[j][:, h, :],
                                    state["ests"][j][:, s, w:],
                                    start=(j == 0),
                                    stop=(j == njt - 1),
                                )

                        def batch(jb, state=state):
                            def go():
                                if jb == 0:
                                    state["y2ps"] = ps_y.tile(
                                        [D + 1, 2, 512], f32, name="y2ps"
                                    )
                                # pv first: its inputs (est from 2 batches
                                # ago) are always ready, so a stalled s_pair
                                # (st-slot recycle waits on ACT) can't
                                # head-of-line-block ready PE work
                                if jb >= 4:
                                    pv_block(jb - 4, state)
                                    pv_block(jb - 3, state)
                                stA = s_pair(jb, state)
                                stB = s_pair(jb + 1, state)
                                state["ests"][jb] = exp_block(jb, stA)
                                state["ests"][jb + 1] = exp_block(jb + 1, stB)

                            return go

                        def tail(state=state, hp=hp):
                            def go():
                                for j_ in range(max(0, njt - 4), njt):
                                    pv_block(j_, state)
                                # epi part 1: denominator row -> bf16 sbuf
                                rsb = rsb_slots[(2 * jq + hp) % 2]
                                nc.vector.tensor_copy(
                                    rsb[D : D + 1, :, :],
                                    state["y2ps"][D : D + 1, :, :],
                                )

                            return go

                        def epi2(state=state, hp=hp):
                            def go():
                                # K=128 bf16 broadcast (full-array mode) ->
                                # DVE reciprocal -> psum-by-sbuf normalize.
                                rsb = rsb_slots[(2 * jq + hp) % 2]
                                y2ps = state["y2ps"]
                                rb_sb = attsmall.tile([D, 2, 512], f32, tag="rbs")
                                for s in range(2):
                                    rb_ps = ps_share.tile(
                                        [D, 512], f32, tag="share", name=f"rb{s}"
                                    )
                                    nc.tensor.matmul(
                                        rb_ps,
                                        ones_sb[:, :],
                                        rsb[:, s, :],
                                        start=True,
                                        stop=True,
                                    )
                                    nc.vector.reciprocal_approx_fast(
                                        rb_sb[:, s, :], rb_ps
                                    )
                                nc.vector.tensor_mul(
                                    y2_t[jq][0:D, hp, :],
                                    y2ps[0:D, 0, :],
                                    rb_sb[:, 0, :],
                                )
                                y_lo = attsmall.tile([D, 512], bf16, tag="ylo")
                                nc.vector.tensor_mul(
                                    y_lo, y2ps[0:D, 1, :], rb_sb[:, 1, :]
                                )
                                nc.gpsimd.dma_start(
                                    y2_t[jq][D:128, hp, :], y_lo
                                )

                            return go

                        for jb in range(0, njt, 2):
                            chunks.append(batch(jb))
                        chunks.append(tail())
                        chunks.append(epi2())

                    for hp in range(2):
                        build_hp(hp)
                    return chunks

                def weave(chunks, fillers):
                    n, m = len(chunks), len(fillers)
                    fi = 0
                    for i, ch in enumerate(chunks):
                        ch()
                        want = (i + 1) * m // n
                        while fi < want:
                            fillers[fi]()
                            fi += 1

                # DMA order matters for the kernel head: consts go first so
                # the PE can run warm-up matmuls (HAM un-throttle) during the
                # x8/wqk8 wait; those are split in interleaved halves so the
                # first qkv group only waits for its own contraction slices.
                nc.sync.dma_start(consts_sb, consts[:, :])
                for t in range(2):
                    nc.sync.dma_start(x8_q[0][:, t : t + 1], x8_in[:, 0, t : t + 1])
                    nc.sync.dma_start(wqk8_sb[:, t : t + 1], wqk8[:, t : t + 1])
                nc.sync.dma_start(x8_q[0][:, 2:], x8_in[:, 0, 2:])
                nc.sync.dma_start(wqk8_sb[:, 2:], wqk8[:, 2:])
                nc.sync.dma_start(x_q[0], x_in[:, 0])
                nc.sync.dma_start(wv_sb, wv[:, :, :])
                nc.sync.dma_start(wp_sb, wp[:, :, :])

                # PE warm-up: ~1us of throwaway matmuls on the consts tile
                # while the first x8/wqk8 chunks are still in flight, so the
                # HAM clock gate opens to 2.4 GHz before the real work lands
                warm_ps = ps_share.tile([128, 256], f32, tag="share")
                for _ in range(4):
                    nc.tensor.matmul(
                        warm_ps, consts_sb[:, 0:128], consts_sb[:, 0:256],
                        start=True, stop=True,
                    )

                # head: interleave stage-0 qkv with row-0 attention so the
                # ACT exp stream (the second-longest engine resource) starts
                # as soon as hp0's q and k are in SBUF instead of after all
                # eight qkv groups
                g0 = stage_groups(0)  # [qk0, qk1, v0, qk2, v1, qk3, v2, v3]
                c0 = attention_chunks(0)
                for step in (g0[0], g0[3], c0[0], g0[1], g0[5], c0[1],
                             g0[2], g0[4], g0[6], g0[7]):
                    step()
                stage_dma(1)
                weave(c0[2:], stage_groups(1))
                # rows 1-2 are woven with the next stage's qkv groups and the
                # previous block's c_proj as PE filler; row 3 (ACT-bound: the
                # longest exp stream) gets stage 3's v groups as extra PE
                # filler, front-loaded so they land before hp0's pv tail
                for tb in (1, 2):
                    g_next = stage_groups(tb + 1)
                    stage_dma(tb + 1)
                    fillers = g_next if tb == 1 else [g_next[i] for i in (0, 1, 3, 5)]
                    fillers += [proj_tile(tb - 1, 4 * (tb - 1) + i)
                                for i in range(4)]
                    weave(attention_chunks(tb), fillers)
                g3v = [stage_groups(3)[i] for i in (2, 4, 6, 7)]
                c3 = attention_chunks(3)
                weave(c3[0:8], g3v)
                weave(c3[8:], [proj_tile(2, 8 + i) for i in range(4)])
                for i in range(4):
                    proj_tile(3, 12 + i)()

    nc.compile()
    return nc


def _get_nc():
    if "nc" not in _CACHE:
        _CACHE["nc"] = _build()
    return _CACHE["nc"]


def make_in_maps(x, w_attn, w_proj):
    import ml_dtypes

    bf16 = ml_dtypes.bfloat16
    fp8 = getattr(ml_dtypes, "float8_e4m3fn", None) or ml_dtypes.float8_e4m3
    x = np.asarray(x, np.float32)
    w_attn = np.asarray(w_attn, np.float32)
    w_proj = np.asarray(w_proj, np.float32)

    # 1 where k <= q (keep), 0 where k > q (causal-masked), twice for s=0/1
    triu01 = np.triu(np.ones((128, 128), np.float32), 0)
    consts = np.concatenate([triu01, triu01], axis=1).astype(bf16)

    in_maps = []
    for c in range(N_CORES):
        b, hg = c // 4, c % 4
        hs = hg * HL * D  # 256 * hg
        xt = np.ascontiguousarray(x[b].T)  # [C, T]
        # [128, tb, KT, 512]: per-partition-contiguous per stage
        x_t = xt.reshape(KT, 128, 4, T // 4).transpose(1, 2, 0, 3)
        x_t = np.ascontiguousarray(x_t)
        x8_t = x_t.reshape(128, 4, KT // 2, 2, T // 4)
        wq = w_attn[hs : hs + HL * D, :]
        wk = w_attn[C + hs : C + hs + HL * D, :]
        wqkt = np.concatenate([wq, wk], 0).T  # [C, 512]
        wqk_t = np.ascontiguousarray(
            wqkt.reshape(KT, 128, 2 * HL * D).transpose(1, 0, 2)
        )
        wqk8_t = (wqk_t * WS).reshape(128, KT // 2, 2, 2 * HL * D)
        wvt = w_attn[2 * C + hs : 2 * C + hs + HL * D, :].T  # [C, 256]
        wv_t = wvt.reshape(KT, 128, HL * D).transpose(1, 0, 2)
        # head-pair stacked rows: [128, HL//2, C]; partition p of pair pr is
        # local feature pr*128 + p (head 2*pr dims then head 2*pr+1 dims)
        wp_t = (
            w_proj[:, hs : hs + HL * D].T.reshape(HL // 2, 128, C).transpose(1, 0, 2)
        )
        in_maps.append(
            {
                "x_in": x_t.astype(bf16),
                "x8_in": x8_t.astype(bf16).astype(fp8),
                "wqk8": wqk8_t.astype(bf16).astype(fp8),
                "wv": np.ascontiguousarray(wv_t).astype(bf16),
                "wp": np.ascontiguousarray(wp_t).astype(bf16),
                "consts": consts,
            }
        )
    return in_maps


def run(in_maps, **kwargs):
    nc = _get_nc()
    return run_bass_kernel_spmd(nc, in_maps, core_ids=list(range(N_CORES)), **kwargs)


def combine(results):
    out = np.zeros((B, T, C), np.float64)
    for c in range(N_CORES):
        out[c // 4] += results[c]["out"].astype(np.float64)
    return out.astype(np.float32)


def kernel(x, w_attn, w_proj):
    res = run(make_in_maps(x, w_attn, w_proj))
    return combine(res.results)


# revision 20
# speedup vs baseline: 1.0341x; 1.0272x over previous
"""Causal self-attention on 8 Trainium2 NeuronCores.

Sharding (batch + head parallel): core c handles batch b = c // 4 and the
4 heads [hg*4, hg*4+4) where hg = c % 4.  Each core computes q/k/v from
column-sliced c_attn weights, full causal attention for its heads, and a
partial c_proj output from the matching row slice of w_proj; the host sums
the 4 partials per batch.

Matmuls run in bf16 (fp32 PSUM accumulate), except the q/k projections
which run in fp8-e4m3 with DoubleRow perf mode (two k-tiles per matmul,
2 elem/cell/cycle); wq/wk are pre-scaled by 32 to clear the e4m3
subnormal range and the 1/(32*32) is folded into the softmax scale.
The two heads of a pair
are computed CONCURRENTLY in the PE array for the S = K^T Q matmuls via
row tiling (K=64 contraction each, tile_position rows 0-63 / 64-127); S
batches of two key blocks keep the PE in 64-row tiling mode back-to-back
so the mode-switch drain is paid once per batch.  The causal mask is a
0/1 triu multiply applied to the exp() output on the (otherwise idle)
GPSIMD engine, so the PE only runs the productive matmul streams; the
softmax denominator is broadcast across partitions with a bf16 ones
matmul (vs f32r in earlier revisions - ~3x faster on the PE).

Emission WEAVES attention chunks with independent PE filler work (the
next stage's qkv matmul groups and c_proj halves) so the ACT exp stream
- the second-busiest engine - runs from ~10us in, and the PE always has
independent work queued behind ACT-dependent chunks (no head-of-line
stalls, HAM clock-gate stays at 2.4 GHz).  The first x stage and wqk are
DMAd in interleaved per-kt chunks so the first qkv matmul only waits for
one 256KB pair instead of the full 2MB.
"""

import sys

if "/opt/trn_rl_repo" not in sys.path:
    sys.path.insert(0, "/opt/trn_rl_repo")

import numpy as np

import concourse.mybir as mybir
from concourse import bacc
from concourse.bass_utils import run_bass_kernel_spmd
from concourse.tile import TileContext

B, T, C = 2, 2048, 1024
H, D = 16, 64
HL = 4  # heads per core
N_CORES = 8
KT = C // 128  # contraction tiles over the embedding dim
WS = 32.0  # fp8 pre-scale on wq/wk (keeps them out of e4m3 subnormals)
SCALE = 1.0 / (8.0 * WS * WS)  # 1/sqrt(D), with the q&k pre-scales folded in

_CACHE = {}


def _build():
    f32 = mybir.dt.float32
    bf16 = mybir.dt.bfloat16
    fp8 = mybir.dt.float8e4
    DR = mybir.MatmulPerfMode.DoubleRow
    nc = bacc.Bacc("TRN2", target_bir_lowering=False, debug=False, num_devices=N_CORES)

    x_in = nc.dram_tensor("x_in", [128, 4, KT, T // 4], bf16, kind="ExternalInput")
    x8_in = nc.dram_tensor(
        "x8_in", [128, 4, KT // 2, 2, T // 4], fp8, kind="ExternalInput"
    )
    wqk8 = nc.dram_tensor(
        "wqk8", [128, KT // 2, 2, 2 * HL * D], fp8, kind="ExternalInput"
    )
    wv = nc.dram_tensor("wv", [128, KT, HL * D], bf16, kind="ExternalInput")
    wp = nc.dram_tensor("wp", [128, HL // 2, C], bf16, kind="ExternalInput")
    # consts: two copies of triu01 (1 where k <= q) for the two s slices
    consts = nc.dram_tensor("consts", [128, 256], bf16, kind="ExternalInput")
    out = nc.dram_tensor("out", [T, C], bf16, kind="ExternalOutput")

    EXP = mybir.ActivationFunctionType.Exp

    with TileContext(nc) as tc:
        with tc.tile_pool(name="persist", bufs=1) as persist:
            # q/k feature-major [d, t]: slot 0/1 = q head-pairs 0/1, 2/3 = k;
            # partitions 0-63 = even head dims, 64-127 = odd head dims.
            qk_t = [
                [persist.tile([128, 512], bf16, name=f"qk{s}_{tb}") for tb in range(4)]
                for s in range(4)
            ]
            # v token-major per 128-token tile; col D holds ones (denominator)
            v_t = [
                persist.tile([128, HL, D + 1], bf16, name=f"v{tt}") for tt in range(16)
            ]
            # head-pair stacked normalized y per 512-token block
            y2_t = [
                persist.tile([128, HL // 2, 512], bf16, name=f"y2{b_}")
                for b_ in range(4)
            ]
            wp_sb = persist.tile([128, HL // 2, C], bf16)
            consts_sb = persist.tile([128, 256], bf16)
            triu2 = consts_sb[:, :].rearrange("p (s q) -> p s q", s=2)
            ones_sb = persist.tile([128, 64], bf16)
            # denominator staging rows: all partitions zero except row D, so
            # the broadcast matmul can contract K=128 (full-array mode, no
            # tiling-mode switch/drain on the PE)
            rsb_slots = [
                persist.tile([128, 2, 512], bf16, name=f"rsb{i}") for i in range(2)
            ]
            for r_ in rsb_slots:
                nc.gpsimd.memset(r_.bitcast(mybir.dt.uint16), 0)
            nc.gpsimd.memset(ones_sb, 1.0)

            for tt in range(16):
                nc.gpsimd.memset(v_t[tt][:, :, D : D + 1], 1.0)

            with (
                tc.tile_pool(name="qkvp", bufs=1) as qkvp,
                tc.tile_pool(name="attp", bufs=10) as attp,
                tc.tile_pool(name="attsmall", bufs=3) as attsmall,
                tc.tile_pool(name="projp", bufs=3) as projp,
                tc.tile_pool(name="ps_st", bufs=2, space="PSUM") as ps_st,
                tc.tile_pool(name="ps_y", bufs=1, space="PSUM") as ps_y,
                tc.tile_pool(name="ps_share", bufs=2, space="PSUM") as ps_share,
            ):
                # quarter-length x buffers, double-buffered across stages
                x_q = [
                    qkvp.tile([128, KT, T // 4], bf16, name=f"x_q{i}") for i in range(2)
                ]
                x8_q = [
                    qkvp.tile([128, KT // 2, 2, T // 4], fp8, name=f"x8_q{i}")
                    for i in range(2)
                ]
                wqk8_sb = qkvp.tile([128, KT // 2, 2, 2 * HL * D], fp8)
                wv_sb = qkvp.tile([128, KT, HL * D], bf16)

                def stage_dma(tb):
                    # stage-major x layout: 8KB contiguous per partition ->
                    # few large DMA descriptors instead of ~1K small ones
                    nc.sync.dma_start(x8_q[tb % 2], x8_in[:, tb])
                    nc.sync.dma_start(x_q[tb % 2], x_in[:, tb])

                def qk_group(tb, jt):
                    def go():
                        # fp8 DoubleRow: two 128-deep k-tiles per matmul at
                        # 2 elem/cell/cycle - ~1.4x the bf16 qk throughput
                        x8 = x8_q[tb % 2]
                        qk_ps = ps_share.tile(
                            [128, 512], f32, tag="share", name="qk_ps"
                        )
                        for t in range(KT // 2):
                            nc.tensor.matmul(
                                qk_ps,
                                wqk8_sb[:, t, :, jt * 128 : (jt + 1) * 128],
                                x8[:, t],
                                start=(t == 0),
                                stop=(t == KT // 2 - 1),
                                perf_mode=DR,
                            )
                        nc.vector.tensor_copy(qk_t[jt][tb], qk_ps)

                    return go

                def v_group(tb, tt2):
                    def go():
                        x_sb = x_q[tb % 2]
                        tt = tb * 4 + tt2
                        v_ps = ps_share.tile(
                            [128, HL * D], f32, tag="share", name="v_ps"
                        )
                        for kt in range(KT):
                            nc.tensor.matmul(
                                v_ps,
                                x_sb[:, kt, tt2 * 128 : (tt2 + 1) * 128],
                                wv_sb[:, kt, :],
                                start=(kt == 0),
                                stop=(kt == KT - 1),
                            )
                        nc.vector.tensor_copy(
                            v_t[tt][:, :, 0:D],
                            v_ps.rearrange("p (h d) -> p h d", h=HL),
                        )

                    return go

                def stage_groups(tb):
                    # zipped so q and k/v land evenly through the row
                    return [qk_group(tb, 0), qk_group(tb, 1), v_group(tb, 0),
                            qk_group(tb, 2), v_group(tb, 1), qk_group(tb, 3),
                            v_group(tb, 2), v_group(tb, 3)]

                def proj_tile(blk, tt):
                    def go():
                        o_sb = projp.tile([128, C], bf16, name="o_sb")
                        off = (tt % 4) * 128
                        for cb in range(2):
                            o_ps = ps_share.tile(
                                [128, 512], f32, tag="share", name="o_ps"
                            )
                            for pr in range(2):
                                nc.tensor.matmul(
                                    o_ps,
                                    y2_t[blk][:, pr, off : off + 128],
                                    wp_sb[:, pr, cb * 512 : (cb + 1) * 512],
                                    start=(pr == 0),
                                    stop=(pr == 1),
                                )
                            # the last block runs after the exp stream ends:
                            # evacuate on ACT and DVE alternately so neither
                            # engine serializes the final matmul stream
                            if blk == 3:
                                if cb == 0:
                                    nc.scalar.copy(o_sb[:, 0:512], o_ps)
                                else:
                                    nc.vector.tensor_copy(o_sb[:, 512:1024], o_ps)
                                nc.sync.dma_start(
                                    out[tt * 128 : (tt + 1) * 128,
                                        cb * 512 : (cb + 1) * 512],
                                    o_sb[:, cb * 512 : (cb + 1) * 512],
                                )
                            else:
                                nc.vector.tensor_copy(
                                    o_sb[:, cb * 512 : (cb + 1) * 512], o_ps
                                )
                        if blk != 3:
                            nc.sync.dma_start(out[tt * 128 : (tt + 1) * 128, :], o_sb)

                    return go

                def attention_chunks(jq):
                    njt = 4 * (jq + 1)
                    chunks = []

                    def build_hp(hp):
                        # separate scope per hp: closures run later (in the
                        # weave), so they must bind this hp's values now
                        h0, h1 = 2 * hp, 2 * hp + 1
                        kslot, qslot = 2 + hp, hp
                        state = {"ests": [None] * njt}

                        def s_pair(j, state):
                            w = max(0, (j - 4 * jq) * 128)
                            st = ps_st.tile([128, 2, 512], f32, name="st")
                            for s, pbase in ((0, 0), (1, 64)):
                                nc.tensor.matmul(
                                    st[:, s, w:],
                                    qk_t[kslot][j // 4][
                                        pbase : pbase + D,
                                        (j % 4) * 128 : (j % 4 + 1) * 128,
                                    ],
                                    qk_t[qslot][jq][pbase : pbase + D, w:],
                                    start=True,
                                    stop=True,
                                    tile_position=(pbase, 0),
                                )
                            return st

                        def exp_block(j, st):
                            w = max(0, (j - 4 * jq) * 128)
                            est = attp.tile(
                                [128, 2, 512], bf16, tag="est", name="est"
                            )
                            nc.scalar.activation(
                                est[:, :, w:], st[:, :, w:], EXP, scale=SCALE
                            )
                            if j >= 4 * jq:
                                # causal mask: zero est where k > q inside the
                                # diagonal 128-block, on the idle GPSIMD
                                nc.gpsimd.tensor_mul(
                                    est[:, :, w : w + 128],
                                    est[:, :, w : w + 128],
                                    triu2,
                                )
                            return est

                        def pv_block(j, state):
                            w = max(0, (j - 4 * jq) * 128)
                            for s, h in ((0, h0), (1, h1)):
                                nc.tensor.matmul(
                                    state["y2ps"][:, s, w:],
                                    v_t[j][:, h, :],
                                    state["ests"][j][:, s, w:],
                                    start=(j == 0),
                                    stop=(j == njt - 1),
                                )

                        def batch(jb, state=state):
                            def go():
                                if jb == 0:
                                    state["y2ps"] = ps_y.tile(
                                        [D + 1, 2, 512], f32, name="y2ps"
                                    )
                                stA = s_pair(jb, state)
                                stB = s_pair(jb + 1, state)
                                state["ests"][jb] = exp_block(jb, stA)
                                state["ests"][jb + 1] = exp_block(jb + 1, stB)
                                if jb >= 4:
                                    pv_block(jb - 4, state)
                                    pv_block(jb - 3, state)

                            return go

                        def tail(state=state, hp=hp):
                            def go():
                                for j_ in range(max(0, njt - 4), njt):
                                    pv_block(j_, state)
                                # epi part 1: denominator row -> bf16 sbuf
                                rsb = rsb_slots[(2 * jq + hp) % 2]
                                nc.vector.tensor_copy(
                                    rsb[D : D + 1, :, :],
                                    state["y2ps"][D : D + 1, :, :],
                                )

                            return go

                        def epi2(state=state, hp=hp):
                            def go():
                                # K=128 bf16 broadcast (full-array mode) ->
                                # DVE reciprocal -> psum-by-sbuf normalize.
                                rsb = rsb_slots[(2 * jq + hp) % 2]
                                y2ps = state["y2ps"]
                                rb_sb = attsmall.tile([D, 2, 512], f32, tag="rbs")
                                for s in range(2):
                                    rb_ps = ps_share.tile(
                                        [D, 512], f32, tag="share", name=f"rb{s}"
                                    )
                                    nc.tensor.matmul(
                                        rb_ps,
                                        ones_sb[:, :],
                                        rsb[:, s, :],
                                        start=True,
                                        stop=True,
                                    )
                                    nc.vector.reciprocal_approx_fast(
                                        rb_sb[:, s, :], rb_ps
                                    )
                                nc.vector.tensor_mul(
                                    y2_t[jq][0:D, hp, :],
                                    y2ps[0:D, 0, :],
                                    rb_sb[:, 0, :],
                                )
                                y_lo = attsmall.tile([D, 512], bf16, tag="ylo")
                                nc.vector.tensor_mul(
                                    y_lo, y2ps[0:D, 1, :], rb_sb[:, 1, :]
                                )
                                nc.gpsimd.dma_start(
                                    y2_t[jq][D:128, hp, :], y_lo
                                )

                            return go

                        for jb in range(0, njt, 2):
                            chunks.append(batch(jb))
                        chunks.append(tail())
                        chunks.append(epi2())

                    for hp in range(2):
                        build_hp(hp)
                    return chunks

                def weave(chunks, fillers):
                    n, m = len(chunks), len(fillers)
                    fi = 0
                    for i, ch in enumerate(chunks):
                        ch()
                        want = (i + 1) * m // n
                        while fi < want:
                            fillers[fi]()
                            fi += 1

                # DMA order matters for the kernel head: consts go first so
                # the PE can run warm-up matmuls (HAM un-throttle) during the
                # x8/wqk8 wait; those are split in interleaved halves so the
                # first qkv group only waits for its own contraction slices.
                HKT = KT // 4
                nc.sync.dma_start(consts_sb, consts[:, :])
                nc.sync.dma_start(x8_q[0][:, 0:HKT], x8_in[:, 0, 0:HKT])
                nc.sync.dma_start(wqk8_sb[:, 0:HKT], wqk8[:, 0:HKT])
                nc.sync.dma_start(x8_q[0][:, HKT:], x8_in[:, 0, HKT:])
                nc.sync.dma_start(wqk8_sb[:, HKT:], wqk8[:, HKT:])
                nc.sync.dma_start(x_q[0], x_in[:, 0])
                nc.sync.dma_start(wv_sb, wv[:, :, :])
                nc.sync.dma_start(wp_sb, wp[:, :, :])

                # PE warm-up: ~2us of throwaway matmuls on the consts tile
                # while the first x8/wqk8 chunks are still in flight, so the
                # HAM clock gate opens to 2.4 GHz before the real work lands
                warm_ps = ps_share.tile([128, 256], f32, tag="share")
                for _ in range(8):
                    nc.tensor.matmul(
                        warm_ps, consts_sb[:, 0:128], consts_sb[:, 0:256],
                        start=True, stop=True,
                    )

                # head: interleave stage-0 qkv with row-0 attention so the
                # ACT exp stream (the second-longest engine resource) starts
                # as soon as hp0's q and k are in SBUF instead of after all
                # eight qkv groups
                g0 = stage_groups(0)  # [qk0, qk1, v0, qk2, v1, qk3, v2, v3]
                c0 = attention_chunks(0)
                for step in (g0[0], g0[3], c0[0], g0[1], g0[5], c0[1],
                             g0[2], g0[4], g0[6], g0[7]):
                    step()
                stage_dma(1)
                weave(c0[2:], stage_groups(1))
                # rows 1-2 are woven with the next stage's qkv groups and the
                # previous block's c_proj as PE filler; row 3 (ACT-bound: the
                # longest exp stream) gets stage 3's v groups as extra PE
                # filler, front-loaded so they land before hp0's pv tail
                for tb in (1, 2):
                    g_next = stage_groups(tb + 1)
                    stage_dma(tb + 1)
                    fillers = g_next if tb == 1 else [g_next[i] for i in (0, 1, 3, 5)]
                    fillers += [proj_tile(tb - 1, 4 * (tb - 1) + i)
                                for i in range(4)]
                    weave(attention_chunks(tb), fillers)
                g3v = [stage_groups(3)[i] for i in (2, 4, 6, 7)]
                c3 = attention_chunks(3)
                weave(c3[0:8], g3v)
                weave(c3[8:], [proj_tile(2, 8 + i) for i in range(4)])
                for i in range(4):
                    proj_tile(3, 12 + i)()

    nc.compile()
    return nc


def _get_nc():
    if "nc" not in _CACHE:
        _CACHE["nc"] = _build()
    return _CACHE["nc"]


def make_in_maps(x, w_attn, w_proj):
    import ml_dtypes

    bf16 = ml_dtypes.bfloat16
    fp8 = getattr(ml_dtypes, "float8_e4m3fn", None) or ml_dtypes.float8_e4m3
    x = np.asarray(x, np.float32)
    w_attn = np.asarray(w_attn, np.float32)
    w_proj = np.asarray(w_proj, np.float32)

    # 1 where k <= q (keep), 0 where k > q (causal-masked), twice for s=0/1
    triu01 = np.triu(np.ones((128, 128), np.float32), 0)
    consts = np.concatenate([triu01, triu01], axis=1).astype(bf16)

    in_maps = []
    for c in range(N_CORES):
        b, hg = c // 4, c % 4
        hs = hg * HL * D  # 256 * hg
        xt = np.ascontiguousarray(x[b].T)  # [C, T]
        # [128, tb, KT, 512]: per-partition-contiguous per stage
        x_t = xt.reshape(KT, 128, 4, T // 4).transpose(1, 2, 0, 3)
        x_t = np.ascontiguousarray(x_t)
        x8_t = x_t.reshape(128, 4, KT // 2, 2, T // 4)
        wq = w_attn[hs : hs + HL * D, :]
        wk = w_attn[C + hs : C + hs + HL * D, :]
        wqkt = np.concatenate([wq, wk], 0).T  # [C, 512]
        wqk_t = np.ascontiguousarray(
            wqkt.reshape(KT, 128, 2 * HL * D).transpose(1, 0, 2)
        )
        wqk8_t = (wqk_t * WS).reshape(128, KT // 2, 2, 2 * HL * D)
        wvt = w_attn[2 * C + hs : 2 * C + hs + HL * D, :].T  # [C, 256]
        wv_t = wvt.reshape(KT, 128, HL * D).transpose(1, 0, 2)
        # head-pair stacked rows: [128, HL//2, C]; partition p of pair pr is
        # local feature pr*128 + p (head 2*pr dims then head 2*pr+1 dims)
        wp_t = (
            w_proj[:, hs : hs + HL * D].T.reshape(HL // 2, 128, C).transpose(1, 0, 2)
        )
        in_maps.append(
            {
                "x_in": x_t.astype(bf16),
                "x8_in": x8_t.astype(bf16).astype(fp8),
                "wqk8": wqk8_t.astype(bf16).astype(fp8),
                "wv": np.ascontiguousarray(wv_t).astype(bf16),
                "wp": np.ascontiguousarray(wp_t).astype(bf16),
                "consts": consts,
            }
        )
    return in_maps


def run(in_maps, **kwargs):
    nc = _get_nc()
    return run_bass_kernel_spmd(nc, in_maps, core_ids=list(range(N_CORES)), **kwargs)


def combine(results):
    out = np.zeros((B, T, C), np.float64)
    for c in range(N_CORES):
        out[c // 4] += results[c]["out"].astype(np.float64)
    return out.astype(np.float32)


def kernel(x, w_attn, w_proj):
    res = run(make_in_maps(x, w_attn, w_proj))
    return combine(res.results)
